# revision 2
# baseline (speedup 1.0000x reference)
"""Data-parallel Trainium2 Bass kernel for the 2-layer FC-LSTM.

B=512 sharded 8x64; weights replicated (bf16); ZERO collectives
(the baseline's 256 AllGathers cost ~31us each = ~8ms serial).
Phases per core:
  B0: layer-0 recurrence with in-loop x-projection (the 96 x-proj
      matmuls of step t+1 fill the PE while step t's cell tail runs).
      Bias b0 rides a constant-1 lane at x[300] (D padded to 384).
      h0_t streamed to DRAM.
  A1: xp1 = W_ih1 @ h0 + b1 for all t as an N=512 GEMM.
  B1: layer-1 recurrence (W_hh1) + mean-pool accumulator + decoder.
SBUF: the big [128, KH*NM*128] weight buffer is ONE shared tag reloaded
per phase (w0h -> w1x -> w1h); bigbuf is shared between A1's h0 blocks
and B1's xp1 blocks. Zero collectives => PE stays at full p-state.
"""
import sys

sys.path.insert(0, "/opt/trn_rl_repo")

import numpy as np

import concourse.bass as bass
import concourse.bacc as bacc
import concourse.mybir as mybir
from concourse import tile
from concourse.bass_utils import run_bass_kernel_spmd

B, T, D, H = 512, 128, 300, 1024
NCORES = 8
BC = B // NCORES          # 64 batch rows per core
DP = 384                  # D padded to 3 chunks of 128; lane 300 = bias-1
KH = H // 128             # 8 hidden K-chunks
NM = 32                   # M-tiles over 4H gate rows

F32 = mybir.dt.float32
BF16 = mybir.dt.bfloat16
AF = mybir.ActivationFunctionType


def _build(t_steps, t_total=None):
    nc = bacc.Bacc("TRN2", target_bir_lowering=False, debug=False, num_devices=NCORES)
    ts = t_steps
    TB = min(8, ts)           # steps per A1/B1 block
    assert ts % TB == 0
    nblk = ts // TB
    NCB = TB * BC             # 512 columns per block at TB=8

    xT = nc.dram_tensor("xT", [ts, 128, 3, BC], BF16, kind="ExternalInput")
    w0x = nc.dram_tensor("w0x", [128, 3 * NM * 128], BF16, kind="ExternalInput")
    w0h = nc.dram_tensor("w0h", [128, KH * NM * 128], BF16, kind="ExternalInput")
    w1x = nc.dram_tensor("w1x", [128, KH * NM * 128], BF16, kind="ExternalInput")
    w1h = nc.dram_tensor("w1h", [128, KH * NM * 128], BF16, kind="ExternalInput")
    b1c = nc.dram_tensor("b1c", [128, NM], F32, kind="ExternalInput")
    wdec = nc.dram_tensor("wdec", [128, KH], F32, kind="ExternalInput")
    out_p = nc.dram_tensor("out_p", [1, BC], F32, kind="ExternalOutput")

    with tile.TileContext(nc) as tc:
        with (
            tc.tile_pool(name="wp", bufs=1) as wp,
            tc.tile_pool(name="xp", bufs=2) as xp,
            tc.tile_pool(name="bb", bufs=2) as bb,
            tc.tile_pool(name="zp", bufs=1) as zp,
            tc.tile_pool(name="cp", bufs=2) as cp,
            tc.tile_pool(name="pp", bufs=1, space="PSUM") as pp,
            tc.tile_pool(name="dp", bufs=1, space="DRAM") as dp,
        ):
            h0_d = dp.tile([KH * 128, ts * BC], BF16, tag="h0d", name="h0d")
            xp1_d = dp.tile([4 * KH * 128, ts * BC], BF16, tag="xp1d", name="xp1d")

            w0x_s = wp.tile([128, 3 * NM * 128], BF16, tag="w0x", name="w0x")
            nc.sync.dma_start(w0x_s[:], w0x.ap())
            # the big weight buffer: w0h during B0, w1x during A1, w1h in B1
            wsh = wp.tile([128, KH * NM * 128], BF16, tag="wsh", name="wsh0")
            nc.sync.dma_start(wsh[:], w0h.ap())

            ps = [
                pp.tile([128, 8, BC], F32, tag=f"ps{i}", name=f"ps{i}")
                for i in range(8)
            ]

            def w_ap(ws, k, m):
                c = (k * NM + m) * 128
                return ws[:, c : c + 128]

            # ================= B0: layer-0 recurrence =================
            h0 = None
            c0 = None
            for t in range(ts):
                xt = xp.tile([128, 3, BC], BF16, tag="xt", name="xt")
                nc.sync.dma_start(xt[:], xT.ap()[t])

                bank = [ps[4 * (t % 2) + g] for g in range(4)]
                for g in range(4):
                    for o in range(8):
                        m = g * 8 + o
                        for kc in range(3):
                            nc.tensor.matmul(
                                bank[g][:, o, :],
                                w_ap(w0x_s, kc, m),
                                xt[:, kc, :],
                                start=(o == 0 and kc == 0),
                                stop=(t == 0 and o == 7 and kc == 2),
                                skip_group_check=True,
                            )
                if t > 0:
                    for g in range(4):
                        for o in range(8):
                            m = g * 8 + o
                            for k in range(KH):
                                nc.tensor.matmul(
                                    bank[g][:, o, :],
                                    w_ap(wsh, k, m),
                                    h0[:, k, :],
                                    start=False,
                                    stop=(o == 7 and k == KH - 1),
                                    skip_group_check=True,
                                )
                zi = zp.tile([128, 8, BC], F32, tag="zi", name="zi")
                zf = zp.tile([128, 8, BC], F32, tag="zf", name="zf")
                zg = zp.tile([128, 8, BC], F32, tag="zg", name="zg")
                zo = zp.tile([128, 8, BC], F32, tag="zo", name="zo")
                nc.scalar.activation(zi[:], bank[0][:], AF.Sigmoid)
                if t > 0:
                    nc.scalar.activation(zf[:], bank[1][:], AF.Sigmoid)
                nc.scalar.activation(zg[:], bank[2][:], AF.Tanh)
                nc.scalar.activation(zo[:], bank[3][:], AF.Sigmoid)
                c0n = cp.tile([128, 8, BC], F32, tag="c0", name="c0")
                if t == 0:
                    nc.vector.tensor_mul(c0n[:], zi[:], zg[:])
                else:
                    ca = zp.tile([128, 8, BC], F32, tag="ca", name="ca")
                    cb = zp.tile([128, 8, BC], F32, tag="cb", name="cb")
                    nc.vector.tensor_mul(ca[:], zf[:], c0[:])
                    nc.vector.tensor_mul(cb[:], zi[:], zg[:])
                    nc.vector.tensor_add(c0n[:], ca[:], cb[:])
                c0 = c0n
                th = zp.tile([128, 8, BC], F32, tag="th", name="th")
                nc.scalar.activation(th[:], c0[:], AF.Tanh)
                h0n = cp.tile([128, 8, BC], BF16, tag="h0", name="h0")
                nc.vector.tensor_mul(h0n[:], zo[:], th[:])
                h0 = h0n
                for k in range(KH):
                    nc.sync.dma_start(
                        h0_d[k * 128 : (k + 1) * 128, t * BC : (t + 1) * BC],
                        h0[:, k, :],
                    )

            # ================= A1: xp1 = W_ih1 @ h0 + b1 =================
            nc.sync.dma_start(wsh[:], w1x.ap())
            b1_s = wp.tile([128, NM], F32, tag="b1c", name="b1c")
            nc.sync.dma_start(b1_s[:], b1c.ap())

            for nb in range(nblk):
                cols = slice(nb * NCB, (nb + 1) * NCB)
                big = bb.tile([128, 4 * KH, NCB], BF16, tag="big", name="hnb")
                for k in range(KH):
                    nc.sync.dma_start(
                        big[:, k, :], h0_d[k * 128 : (k + 1) * 128, cols]
                    )
                for m in range(NM):
                    psA = ps[m % 8]
                    for k in range(KH):
                        nc.tensor.matmul(
                            psA[:],
                            w_ap(wsh, k, m),
                            big[:, k, :],
                            start=(k == 0),
                            stop=(k == KH - 1),
                        )
                    xs = zp.tile([128, TB, BC], BF16, tag=f"xs{m % 2}", name="xs")
                    nc.scalar.activation(
                        xs[:], psA[:, 0:TB, :], AF.Identity, bias=b1_s[:, m : m + 1]
                    )
                    nc.sync.dma_start(
                        xp1_d[m * 128 : (m + 1) * 128, cols], xs[:]
                    )

            # ================= B1: layer-1 recurrence =================
            nc.sync.dma_start(wsh[:], w1h.ap())
            acc = None
            h1 = None
            c1 = None
            for blk in range(nblk):
                cols = slice(blk * NCB, (blk + 1) * NCB)
                big = bb.tile([128, 4 * KH, NCB], BF16, tag="big", name="x1b")
                for m in range(NM):
                    nc.sync.dma_start(
                        big[:, m, :], xp1_d[m * 128 : (m + 1) * 128, cols]
                    )
                for i in range(TB):
                    t = blk * TB + i
                    bank = [ps[4 * (t % 2) + g] for g in range(4)]
                    if t > 0:
                        for g in range(4):
                            for o in range(8):
                                m = g * 8 + o
                                for k in range(KH):
                                    nc.tensor.matmul(
                                        bank[g][:, o, :],
                                        w_ap(wsh, k, m),
                                        h1[:, k, :],
                                        start=(o == 0 and k == 0),
                                        stop=(o == 7 and k == KH - 1),
                                        skip_group_check=True,
                                    )
                    x1g = [
                        big[:, g * 8 : (g + 1) * 8, i * BC : (i + 1) * BC]
                        for g in range(4)
                    ]
                    yi = zp.tile([128, 8, BC], F32, tag="zi", name="yi")
                    yf = zp.tile([128, 8, BC], F32, tag="zf", name="yf")
                    yg = zp.tile([128, 8, BC], F32, tag="zg", name="yg")
                    yo = zp.tile([128, 8, BC], F32, tag="zo", name="yo")
                    if t == 0:
                        nc.scalar.activation(yi[:], x1g[0], AF.Sigmoid)
                        nc.scalar.activation(yg[:], x1g[2], AF.Tanh)
                        nc.scalar.activation(yo[:], x1g[3], AF.Sigmoid)
                    else:
                        pi = zp.tile([128, 8, BC], F32, tag="pi", name="pi")
                        pf = zp.tile([128, 8, BC], F32, tag="pf", name="pf")
                        pg = zp.tile([128, 8, BC], F32, tag="pg", name="pg")
                        po = zp.tile([128, 8, BC], F32, tag="po", name="po")
                        nc.vector.tensor_add(pi[:], bank[0][:], x1g[0])
                        nc.vector.tensor_add(pf[:], bank[1][:], x1g[1])
                        nc.vector.tensor_add(pg[:], bank[2][:], x1g[2])
                        nc.vector.tensor_add(po[:], bank[3][:], x1g[3])
                        nc.scalar.activation(yi[:], pi[:], AF.Sigmoid)
                        nc.scalar.activation(yf[:], pf[:], AF.Sigmoid)
                        nc.scalar.activation(yg[:], pg[:], AF.Tanh)
                        nc.scalar.activation(yo[:], po[:], AF.Sigmoid)
                    c1n = cp.tile([128, 8, BC], F32, tag="c1", name="c1")
                    if t == 0:
                        nc.vector.tensor_mul(c1n[:], yi[:], yg[:])
                    else:
                        da = zp.tile([128, 8, BC], F32, tag="ca", name="da")
                        db = zp.tile([128, 8, BC], F32, tag="cb", name="db")
                        nc.vector.tensor_mul(da[:], yf[:], c1[:])
                        nc.vector.tensor_mul(db[:], yi[:], yg[:])
                        nc.vector.tensor_add(c1n[:], da[:], db[:])
                    c1 = c1n
                    th1 = zp.tile([128, 8, BC], F32, tag="th", name="th1")
                    nc.scalar.activation(th1[:], c1[:], AF.Tanh)
                    h1n = cp.tile([128, 8, BC], BF16, tag="h1", name="h1")
                    nc.vector.tensor_mul(h1n[:], yo[:], th1[:])
                    h1 = h1n
                    accn = cp.tile([128, 8, BC], F32, tag="acc", name="acc")
                    if t == 0:
                        nc.vector.tensor_copy(accn[:], h1[:])
                    else:
                        nc.vector.tensor_add(accn[:], acc[:], h1[:])
                    acc = accn

            # ================= decoder =================
            wd_s = wp.tile([128, KH], F32, tag="wdec", name="wdec")
            nc.sync.dma_start(wd_s[:], wdec.ap())
            accf = zp.tile([128, 8, BC], F32, tag="pi", name="accf")
            nc.vector.tensor_copy(accf[:], acc[:])
            psd = ps[0]
            for k in range(KH):
                nc.tensor.matmul(
                    psd[0:1, 0, :],
                    wd_s[:, k : k + 1],
                    accf[:, k, :],
                    start=(k == 0),
                    stop=(k == KH - 1),
                )
            outt = zp.tile([1, BC], F32, tag="outt", name="outt")
            nc.scalar.copy(outt[:], psd[0:1, 0, :])
            nc.sync.dma_start(out_p.ap(), outt[:])

    nc.compile()
    return nc


def _pack_w(W, b=None, kdim_pad=None):
    """Pack [4H, K] weights into lhsT layout [128, nk*NM*128] bf16.

    Column ((k*NM)+m)*128 + mc holds W[m*128 + mc, k*128 + p] at
    partition p. If b is given, the lane at input index D (=300) carries it.
    """
    import ml_dtypes

    K = kdim_pad if kdim_pad is not None else W.shape[1]
    nk = K // 128
    Wp = np.zeros((4 * H, K), np.float32)
    Wp[:, : W.shape[1]] = W
    if b is not None:
        Wp[:, D] = b
    arr = np.zeros((128, nk * NM * 128), np.float32)
    for k in range(nk):
        for m in range(NM):
            c = (k * NM + m) * 128
            arr[:, c : c + 128] = Wp[m * 128 : (m + 1) * 128, k * 128 : (k + 1) * 128].T
    return arr.astype(ml_dtypes.bfloat16)


def _prep_inputs(x, W_ih0, W_hh0, b_ih0, b_hh0, W_ih1, W_hh1, b_ih1, b_hh1, W_dec, t_steps):
    import ml_dtypes

    ts = t_steps
    b0 = (b_ih0 + b_hh0).astype(np.float32)
    b1 = (b_ih1 + b_hh1).astype(np.float32)

    w0x_p = _pack_w(np.asarray(W_ih0, np.float32), b=b0, kdim_pad=DP)
    w0h_p = _pack_w(np.asarray(W_hh0, np.float32))
    w1x_p = _pack_w(np.asarray(W_ih1, np.float32))
    w1h_p = _pack_w(np.asarray(W_hh1, np.float32))
    b1c = np.zeros((128, NM), np.float32)
    for m in range(NM):
        b1c[:, m] = b1[m * 128 : (m + 1) * 128]
    wd = np.zeros((128, KH), np.float32)
    for k in range(KH):
        wd[:, k] = np.asarray(W_dec, np.float32)[0, k * 128 : (k + 1) * 128] / np.float32(ts)

    in_maps = []
    for c in range(NCORES):
        xc = np.asarray(x, np.float32)[c * BC : (c + 1) * BC, :ts, :]  # [BC, ts, D]
        xt = np.zeros((ts, DP, BC), np.float32)
        xt[:, :D, :] = np.transpose(xc, (1, 2, 0))
        xt[:, D, :] = 1.0
        # [ts, DP, BC] -> [ts, 128, 3, BC]
        xr = np.ascontiguousarray(
            xt.reshape(ts, 3, 128, BC).transpose(0, 2, 1, 3)
        )
        in_maps.append(
            {
                "xT": xr.astype(ml_dtypes.bfloat16),
                "w0x": w0x_p,
                "w0h": w0h_p,
                "w1x": w1x_p,
                "w1h": w1h_p,
                "b1c": b1c,
                "wdec": wd,
            }
        )
    return in_maps


def _run(inputs, t_steps, **spmd_kwargs):
    nc = _build(t_steps)
    in_maps = _prep_inputs(
        inputs["x"], inputs["W_ih0"], inputs["W_hh0"], inputs["b_ih0"], inputs["b_hh0"],
        inputs["W_ih1"], inputs["W_hh1"], inputs["b_ih1"], inputs["b_hh1"], inputs["W_dec"],
        t_steps,
    )
    res = run_bass_kernel_spmd(nc, in_maps, core_ids=list(range(NCORES)), **spmd_kwargs)
    parts = [res.results[c]["out_p"][0] for c in range(NCORES)]
    out = (np.concatenate(parts) + inputs["b_dec"][0]).astype(np.float32).reshape(B, 1)
    return out, res


def _sim_one_core(inputs, t_steps, core_id=0):
    """Validate numerics on CoreSim (no hardware, no walrus)."""
    from concourse.bass_interp import CoreSim

    nc = _build(t_steps)
    in_maps = _prep_inputs(
        inputs["x"], inputs["W_ih0"], inputs["W_hh0"], inputs["b_ih0"], inputs["b_hh0"],
        inputs["W_ih1"], inputs["W_hh1"], inputs["b_ih1"], inputs["b_hh1"], inputs["W_dec"],
        t_steps,
    )
    sim = CoreSim(nc, require_finite=False, require_nnan=False)
    for name, val in in_maps[core_id].items():
        view = sim.tensor(name)
        view[:] = val
    sim.simulate()
    out = np.array(sim.tensor("out_p"))
    return out[0] + np.asarray(inputs["b_dec"], np.float32)[0]


def _bench(inputs, t_steps, n_timed=30):
    import time
    import jax
    from jax.experimental.shard_map import shard_map
    from jax.sharding import Mesh, PartitionSpec, NamedSharding
    from concourse import bass2jax
    from concourse import mybir as _mybir

    nc = _build(t_steps)
    in_maps = _prep_inputs(
        inputs["x"], inputs["W_ih0"], inputs["W_hh0"], inputs["b_ih0"], inputs["b_hh0"],
        inputs["W_ih1"], inputs["W_hh1"], inputs["b_ih1"], inputs["b_hh1"], inputs["W_dec"],
        t_steps,
    )
    bass2jax.install_neuronx_cc_hook()

    partition_name = nc.partition_id_tensor.name if nc.partition_id_tensor else None
    in_names, out_names, out_avals, zero_outs = [], [], [], []
    for alloc in nc.m.functions[0].allocations:
        if not isinstance(alloc, _mybir.MemoryLocationSet):
            continue
        name = alloc.memorylocations[0].name
        if alloc.kind == "ExternalInput":
            if name != partition_name:
                in_names.append(name)
        elif alloc.kind == "ExternalOutput":
            shape = tuple(alloc.tensor_shape)
            dtype = _mybir.dt.np(alloc.dtype)
            out_names.append(name)
            out_avals.append(jax.core.ShapedArray(shape, dtype))
            zero_outs.append(np.zeros(shape, dtype))
    n_params = len(in_names)
    all_in_names = list(in_names) + list(out_names)
    if partition_name is not None:
        all_in_names.append(partition_name)

    def _body(*args):
        operands = list(args)
        if partition_name is not None:
            operands.append(bass2jax.partition_id_tensor())
        outs = bass2jax._bass_exec_p.bind(
            *operands,
            out_avals=tuple(out_avals),
            in_names=tuple(all_in_names),
            out_names=tuple(out_names),
            lowering_input_output_aliases=(),
            sim_require_finite=True,
            sim_require_nnan=True,
            nc=nc,
        )
        return tuple(outs)

    devices = jax.devices()[:NCORES]
    mesh = Mesh(np.asarray(devices), ("core",))
    spec = PartitionSpec("core")
    n_outs = len(out_names)
    sharded = jax.jit(
        shard_map(_body, mesh=mesh, in_specs=(spec,) * (n_params + n_outs),
                  out_specs=(spec,) * n_outs, check_rep=False),
        keep_unused=True,
    )
    sharding = NamedSharding(mesh, spec)
    dev_args = []
    for name in in_names:
        cat = np.concatenate([np.asarray(in_maps[c][name]) for c in range(NCORES)], axis=0)
        dev_args.append(jax.device_put(cat, sharding))
    for z in zero_outs:
        cat = np.zeros((NCORES * z.shape[0], *z.shape[1:]), z.dtype)
        dev_args.append(jax.device_put(cat, sharding))

    out_arrs = sharded(*dev_args)
    jax.block_until_ready(out_arrs)
    times = []
    for _ in range(n_timed):
        t0 = time.perf_counter()
        out_arrs = sharded(*dev_args)
        jax.block_until_ready(out_arrs)
        times.append(time.perf_counter() - t0)

    parts = np.asarray(out_arrs[out_names.index("out_p")]).reshape(NCORES, BC)
    out = (parts.reshape(B) + inputs["b_dec"][0]).astype(np.float32).reshape(B, 1)
    return out, times


def kernel(**inputs):
    out, _ = _run(inputs, T)
    return out


# revision 3
# speedup vs baseline: 1.2806x; 1.2806x over previous
"""Data-parallel Trainium2 Bass kernel for the 2-layer FC-LSTM.

B=512 sharded 8x64; weights replicated (bf16); ZERO collectives
(the baseline's 256 AllGathers cost ~31us each = ~8ms serial).
Phases per core:
  B0: layer-0 recurrence with in-loop x-projection (the 96 x-proj
      matmuls of step t+1 fill the PE while step t's cell tail runs).
      Bias b0 rides a constant-1 lane at x[300] (D padded to 384).
      h0_t streamed to DRAM.
  A1: xp1 = W_ih1 @ h0 + b1 for all t as an N=512 GEMM.
  B1: layer-1 recurrence (W_hh1) + mean-pool accumulator + decoder.
SBUF: the big [128, KH*NM*128] weight buffer is ONE shared tag reloaded
per phase (w0h -> w1x -> w1h); bigbuf is shared between A1's h0 blocks
and B1's xp1 blocks. Zero collectives => PE stays at full p-state.
"""
import sys

sys.path.insert(0, "/opt/trn_rl_repo")

import numpy as np

import concourse.bass as bass
import concourse.bacc as bacc
import concourse.mybir as mybir
from concourse import tile
from concourse.bass_utils import run_bass_kernel_spmd

B, T, D, H = 512, 128, 300, 1024
NCORES = 8
BC = B // NCORES          # 64 batch rows per core
DP = 384                  # D padded to 3 chunks of 128; lane 300 = bias-1
KH = H // 128             # 8 hidden K-chunks
NM = 32                   # M-tiles over 4H gate rows

F32 = mybir.dt.float32
BF16 = mybir.dt.bfloat16
AF = mybir.ActivationFunctionType


def _build(t_steps, t_total=None):
    nc = bacc.Bacc("TRN2", target_bir_lowering=False, debug=False, num_devices=NCORES)
    ts = t_steps
    TB = min(8, ts)           # steps per A1/B1 block
    assert ts % TB == 0
    nblk = ts // TB
    NCB = TB * BC             # 512 columns per block at TB=8

    xT = nc.dram_tensor("xT", [ts, 128, 3, BC], BF16, kind="ExternalInput")
    w0x = nc.dram_tensor("w0x", [128, 3 * NM * 128], BF16, kind="ExternalInput")
    w0h = nc.dram_tensor("w0h", [128, KH * NM * 128], BF16, kind="ExternalInput")
    w1x = nc.dram_tensor("w1x", [128, KH * NM * 128], BF16, kind="ExternalInput")
    w1h = nc.dram_tensor("w1h", [128, KH * NM * 128], BF16, kind="ExternalInput")
    b1c = nc.dram_tensor("b1c", [128, NM], F32, kind="ExternalInput")
    wdec = nc.dram_tensor("wdec", [128, KH], F32, kind="ExternalInput")
    out_p = nc.dram_tensor("out_p", [1, BC], F32, kind="ExternalOutput")

    with tile.TileContext(nc) as tc:
        with (
            tc.tile_pool(name="wp", bufs=1) as wp,
            tc.tile_pool(name="xp", bufs=2) as xp,
            tc.tile_pool(name="bb", bufs=2) as bb,
            tc.tile_pool(name="zp", bufs=1) as zp,
            tc.tile_pool(name="cp", bufs=2) as cp,
            tc.tile_pool(name="pp", bufs=1, space="PSUM") as pp,
            tc.tile_pool(name="dp", bufs=1, space="DRAM") as dp,
        ):
            h0_d = dp.tile([128, KH, ts * BC], BF16, tag="h0d", name="h0d")
            xp1_d = dp.tile([4 * KH * 128, ts * BC], BF16, tag="xp1d", name="xp1d")

            w0x_s = wp.tile([128, 3 * NM * 128], BF16, tag="w0x", name="w0x")
            nc.sync.dma_start(w0x_s[:], w0x.ap())
            # the big weight buffer: w0h during B0, w1x during A1, w1h in B1
            wsh = wp.tile([128, KH * NM * 128], BF16, tag="wsh", name="wsh0")
            nc.sync.dma_start(wsh[:], w0h.ap())

            ps = [
                pp.tile([128, 8, BC], F32, tag=f"ps{i}", name=f"ps{i}")
                for i in range(8)
            ]

            def w_ap(ws, k, m):
                c = (k * NM + m) * 128
                return ws[:, c : c + 128]

            # ================= B0: layer-0 recurrence =================
            h0 = None
            c0 = None
            for t in range(ts):
                xt = xp.tile([128, 3, BC], BF16, tag="xt", name="xt")
                nc.sync.dma_start(xt[:], xT.ap()[t])

                bank = [ps[4 * (t % 2) + g] for g in range(4)]
                for g in range(4):
                    for o in range(8):
                        m = g * 8 + o
                        for kc in range(3):
                            nc.tensor.matmul(
                                bank[g][:, o, :],
                                w_ap(w0x_s, kc, m),
                                xt[:, kc, :],
                                start=(o == 0 and kc == 0),
                                stop=(t == 0 and o == 7 and kc == 2),
                                skip_group_check=True,
                            )
                if t > 0:
                    for g in range(4):
                        for o in range(8):
                            m = g * 8 + o
                            for k in range(KH):
                                nc.tensor.matmul(
                                    bank[g][:, o, :],
                                    w_ap(wsh, k, m),
                                    h0[:, k, :],
                                    start=False,
                                    stop=(o == 7 and k == KH - 1),
                                    skip_group_check=True,
                                )
                zi = zp.tile([128, 8, BC], F32, tag="zi", name="zi")
                zf = zp.tile([128, 8, BC], F32, tag="zf", name="zf")
                zg = zp.tile([128, 8, BC], F32, tag="zg", name="zg")
                zo = zp.tile([128, 8, BC], F32, tag="zo", name="zo")
                nc.scalar.activation(zi[:], bank[0][:], AF.Sigmoid)
                if t > 0:
                    nc.scalar.activation(zf[:], bank[1][:], AF.Sigmoid)
                nc.scalar.activation(zg[:], bank[2][:], AF.Tanh)
                nc.scalar.activation(zo[:], bank[3][:], AF.Sigmoid)
                c0n = cp.tile([128, 8, BC], F32, tag="c0", name="c0")
                if t == 0:
                    nc.vector.tensor_mul(c0n[:], zi[:], zg[:])
                else:
                    ca = zp.tile([128, 8, BC], F32, tag="ca", name="ca")
                    cb = zp.tile([128, 8, BC], F32, tag="cb", name="cb")
                    nc.vector.tensor_mul(ca[:], zf[:], c0[:])
                    nc.vector.tensor_mul(cb[:], zi[:], zg[:])
                    nc.vector.tensor_add(c0n[:], ca[:], cb[:])
                c0 = c0n
                th = zp.tile([128, 8, BC], F32, tag="th", name="th")
                nc.scalar.activation(th[:], c0[:], AF.Tanh)
                h0n = cp.tile([128, 8, BC], BF16, tag="h0", name="h0")
                nc.vector.tensor_mul(h0n[:], zo[:], th[:])
                h0 = h0n
                nc.sync.dma_start(h0_d[:, :, t * BC : (t + 1) * BC], h0[:])

            # ================= A1: xp1 = W_ih1 @ h0 + b1 =================
            nc.sync.dma_start(wsh[:], w1x.ap())
            b1_s = wp.tile([128, NM], F32, tag="b1c", name="b1c")
            nc.sync.dma_start(b1_s[:], b1c.ap())

            for nb in range(nblk):
                cols = slice(nb * NCB, (nb + 1) * NCB)
                big = bb.tile([128, 4 * KH, NCB], BF16, tag="big", name="hnb")
                for k in range(KH):
                    nc.sync.dma_start(big[:, k, :], h0_d[:, k, cols])
                for m in range(NM):
                    psA = ps[m % 8]
                    for k in range(KH):
                        nc.tensor.matmul(
                            psA[:],
                            w_ap(wsh, k, m),
                            big[:, k, :],
                            start=(k == 0),
                            stop=(k == KH - 1),
                        )
                    xs = zp.tile([128, TB, BC], BF16, tag=f"xs{m % 2}", name="xs")
                    nc.scalar.activation(
                        xs[:], psA[:, 0:TB, :], AF.Identity, bias=b1_s[:, m : m + 1]
                    )
                    nc.sync.dma_start(
                        xp1_d[m * 128 : (m + 1) * 128, cols], xs[:]
                    )

            # ================= B1: layer-1 recurrence =================
            nc.sync.dma_start(wsh[:], w1h.ap())
            acc = None
            h1 = None
            c1 = None
            for blk in range(nblk):
                cols = slice(blk * NCB, (blk + 1) * NCB)
                big = bb.tile([128, 4 * KH, NCB], BF16, tag="big", name="x1b")
                for m in range(NM):
                    nc.sync.dma_start(
                        big[:, m, :], xp1_d[m * 128 : (m + 1) * 128, cols]
                    )
                for i in range(TB):
                    t = blk * TB + i
                    bank = [ps[4 * (t % 2) + g] for g in range(4)]
                    if t > 0:
                        for g in range(4):
                            for o in range(8):
                                m = g * 8 + o
                                for k in range(KH):
                                    nc.tensor.matmul(
                                        bank[g][:, o, :],
                                        w_ap(wsh, k, m),
                                        h1[:, k, :],
                                        start=(o == 0 and k == 0),
                                        stop=(o == 7 and k == KH - 1),
                                        skip_group_check=True,
                                    )
                    x1g = [
                        big[:, g * 8 : (g + 1) * 8, i * BC : (i + 1) * BC]
                        for g in range(4)
                    ]
                    yi = zp.tile([128, 8, BC], F32, tag="zi", name="yi")
                    yf = zp.tile([128, 8, BC], F32, tag="zf", name="yf")
                    yg = zp.tile([128, 8, BC], F32, tag="zg", name="yg")
                    yo = zp.tile([128, 8, BC], F32, tag="zo", name="yo")
                    if t == 0:
                        nc.scalar.activation(yi[:], x1g[0], AF.Sigmoid)
                        nc.scalar.activation(yg[:], x1g[2], AF.Tanh)
                        nc.scalar.activation(yo[:], x1g[3], AF.Sigmoid)
                    else:
                        pi = zp.tile([128, 8, BC], F32, tag="pi", name="pi")
                        pf = zp.tile([128, 8, BC], F32, tag="pf", name="pf")
                        pg = zp.tile([128, 8, BC], F32, tag="pg", name="pg")
                        po = zp.tile([128, 8, BC], F32, tag="po", name="po")
                        nc.vector.tensor_add(pi[:], bank[0][:], x1g[0])
                        nc.vector.tensor_add(pf[:], bank[1][:], x1g[1])
                        nc.vector.tensor_add(pg[:], bank[2][:], x1g[2])
                        nc.vector.tensor_add(po[:], bank[3][:], x1g[3])
                        nc.scalar.activation(yi[:], pi[:], AF.Sigmoid)
                        nc.scalar.activation(yf[:], pf[:], AF.Sigmoid)
                        nc.scalar.activation(yg[:], pg[:], AF.Tanh)
                        nc.scalar.activation(yo[:], po[:], AF.Sigmoid)
                    c1n = cp.tile([128, 8, BC], F32, tag="c1", name="c1")
                    if t == 0:
                        nc.vector.tensor_mul(c1n[:], yi[:], yg[:])
                    else:
                        da = zp.tile([128, 8, BC], F32, tag="ca", name="da")
                        db = zp.tile([128, 8, BC], F32, tag="cb", name="db")
                        nc.vector.tensor_mul(da[:], yf[:], c1[:])
                        nc.vector.tensor_mul(db[:], yi[:], yg[:])
                        nc.vector.tensor_add(c1n[:], da[:], db[:])
                    c1 = c1n
                    th1 = zp.tile([128, 8, BC], F32, tag="th", name="th1")
                    nc.scalar.activation(th1[:], c1[:], AF.Tanh)
                    h1n = cp.tile([128, 8, BC], BF16, tag="h1", name="h1")
                    nc.vector.tensor_mul(h1n[:], yo[:], th1[:])
                    h1 = h1n
                    accn = cp.tile([128, 8, BC], F32, tag="acc", name="acc")
                    if t == 0:
                        nc.vector.tensor_copy(accn[:], h1[:])
                    else:
                        nc.vector.tensor_add(accn[:], acc[:], h1[:])
                    acc = accn

            # ================= decoder =================
            wd_s = wp.tile([128, KH], F32, tag="wdec", name="wdec")
            nc.sync.dma_start(wd_s[:], wdec.ap())
            accf = zp.tile([128, 8, BC], F32, tag="pi", name="accf")
            nc.vector.tensor_copy(accf[:], acc[:])
            psd = ps[0]
            for k in range(KH):
                nc.tensor.matmul(
                    psd[0:1, 0, :],
                    wd_s[:, k : k + 1],
                    accf[:, k, :],
                    start=(k == 0),
                    stop=(k == KH - 1),
                )
            outt = zp.tile([1, BC], F32, tag="outt", name="outt")
            nc.scalar.copy(outt[:], psd[0:1, 0, :])
            nc.sync.dma_start(out_p.ap(), outt[:])

    nc.compile()
    return nc


def _pack_w(W, b=None, kdim_pad=None):
    """Pack [4H, K] weights into lhsT layout [128, nk*NM*128] bf16.

    Column ((k*NM)+m)*128 + mc holds W[m*128 + mc, k*128 + p] at
    partition p. If b is given, the lane at input index D (=300) carries it.
    """
    import ml_dtypes

    K = kdim_pad if kdim_pad is not None else W.shape[1]
    nk = K // 128
    Wp = np.zeros((4 * H, K), np.float32)
    Wp[:, : W.shape[1]] = W
    if b is not None:
        Wp[:, D] = b
    arr = np.zeros((128, nk * NM * 128), np.float32)
    for k in range(nk):
        for m in range(NM):
            c = (k * NM + m) * 128
            arr[:, c : c + 128] = Wp[m * 128 : (m + 1) * 128, k * 128 : (k + 1) * 128].T
    return arr.astype(ml_dtypes.bfloat16)


def _prep_inputs(x, W_ih0, W_hh0, b_ih0, b_hh0, W_ih1, W_hh1, b_ih1, b_hh1, W_dec, t_steps):
    import ml_dtypes

    ts = t_steps
    b0 = (b_ih0 + b_hh0).astype(np.float32)
    b1 = (b_ih1 + b_hh1).astype(np.float32)

    w0x_p = _pack_w(np.asarray(W_ih0, np.float32), b=b0, kdim_pad=DP)
    w0h_p = _pack_w(np.asarray(W_hh0, np.float32))
    w1x_p = _pack_w(np.asarray(W_ih1, np.float32))
    w1h_p = _pack_w(np.asarray(W_hh1, np.float32))
    b1c = np.zeros((128, NM), np.float32)
    for m in range(NM):
        b1c[:, m] = b1[m * 128 : (m + 1) * 128]
    wd = np.zeros((128, KH), np.float32)
    for k in range(KH):
        wd[:, k] = np.asarray(W_dec, np.float32)[0, k * 128 : (k + 1) * 128] / np.float32(ts)

    in_maps = []
    for c in range(NCORES):
        xc = np.asarray(x, np.float32)[c * BC : (c + 1) * BC, :ts, :]  # [BC, ts, D]
        xt = np.zeros((ts, DP, BC), np.float32)
        xt[:, :D, :] = np.transpose(xc, (1, 2, 0))
        xt[:, D, :] = 1.0
        # [ts, DP, BC] -> [ts, 128, 3, BC]
        xr = np.ascontiguousarray(
            xt.reshape(ts, 3, 128, BC).transpose(0, 2, 1, 3)
        )
        in_maps.append(
            {
                "xT": xr.astype(ml_dtypes.bfloat16),
                "w0x": w0x_p,
                "w0h": w0h_p,
                "w1x": w1x_p,
                "w1h": w1h_p,
                "b1c": b1c,
                "wdec": wd,
            }
        )
    return in_maps


def _run(inputs, t_steps, **spmd_kwargs):
    nc = _build(t_steps)
    in_maps = _prep_inputs(
        inputs["x"], inputs["W_ih0"], inputs["W_hh0"], inputs["b_ih0"], inputs["b_hh0"],
        inputs["W_ih1"], inputs["W_hh1"], inputs["b_ih1"], inputs["b_hh1"], inputs["W_dec"],
        t_steps,
    )
    res = run_bass_kernel_spmd(nc, in_maps, core_ids=list(range(NCORES)), **spmd_kwargs)
    parts = [res.results[c]["out_p"][0] for c in range(NCORES)]
    out = (np.concatenate(parts) + inputs["b_dec"][0]).astype(np.float32).reshape(B, 1)
    return out, res


def _sim_one_core(inputs, t_steps, core_id=0):
    """Validate numerics on CoreSim (no hardware, no walrus)."""
    from concourse.bass_interp import CoreSim

    nc = _build(t_steps)
    in_maps = _prep_inputs(
        inputs["x"], inputs["W_ih0"], inputs["W_hh0"], inputs["b_ih0"], inputs["b_hh0"],
        inputs["W_ih1"], inputs["W_hh1"], inputs["b_ih1"], inputs["b_hh1"], inputs["W_dec"],
        t_steps,
    )
    sim = CoreSim(nc, require_finite=False, require_nnan=False)
    for name, val in in_maps[core_id].items():
        view = sim.tensor(name)
        view[:] = val
    sim.simulate()
    out = np.array(sim.tensor("out_p"))
    return out[0] + np.asarray(inputs["b_dec"], np.float32)[0]


def _bench(inputs, t_steps, n_timed=30):
    import time
    import jax
    from jax.experimental.shard_map import shard_map
    from jax.sharding import Mesh, PartitionSpec, NamedSharding
    from concourse import bass2jax
    from concourse import mybir as _mybir

    nc = _build(t_steps)
    in_maps = _prep_inputs(
        inputs["x"], inputs["W_ih0"], inputs["W_hh0"], inputs["b_ih0"], inputs["b_hh0"],
        inputs["W_ih1"], inputs["W_hh1"], inputs["b_ih1"], inputs["b_hh1"], inputs["W_dec"],
        t_steps,
    )
    bass2jax.install_neuronx_cc_hook()

    partition_name = nc.partition_id_tensor.name if nc.partition_id_tensor else None
    in_names, out_names, out_avals, zero_outs = [], [], [], []
    for alloc in nc.m.functions[0].allocations:
        if not isinstance(alloc, _mybir.MemoryLocationSet):
            continue
        name = alloc.memorylocations[0].name
        if alloc.kind == "ExternalInput":
            if name != partition_name:
                in_names.append(name)
        elif alloc.kind == "ExternalOutput":
            shape = tuple(alloc.tensor_shape)
            dtype = _mybir.dt.np(alloc.dtype)
            out_names.append(name)
            out_avals.append(jax.core.ShapedArray(shape, dtype))
            zero_outs.append(np.zeros(shape, dtype))
    n_params = len(in_names)
    all_in_names = list(in_names) + list(out_names)
    if partition_name is not None:
        all_in_names.append(partition_name)

    def _body(*args):
        operands = list(args)
        if partition_name is not None:
            operands.append(bass2jax.partition_id_tensor())
        outs = bass2jax._bass_exec_p.bind(
            *operands,
            out_avals=tuple(out_avals),
            in_names=tuple(all_in_names),
            out_names=tuple(out_names),
            lowering_input_output_aliases=(),
            sim_require_finite=True,
            sim_require_nnan=True,
            nc=nc,
        )
        return tuple(outs)

    devices = jax.devices()[:NCORES]
    mesh = Mesh(np.asarray(devices), ("core",))
    spec = PartitionSpec("core")
    n_outs = len(out_names)
    sharded = jax.jit(
        shard_map(_body, mesh=mesh, in_specs=(spec,) * (n_params + n_outs),
                  out_specs=(spec,) * n_outs, check_rep=False),
        keep_unused=True,
    )
    sharding = NamedSharding(mesh, spec)
    dev_args = []
    for name in in_names:
        cat = np.concatenate([np.asarray(in_maps[c][name]) for c in range(NCORES)], axis=0)
        dev_args.append(jax.device_put(cat, sharding))
    for z in zero_outs:
        cat = np.zeros((NCORES * z.shape[0], *z.shape[1:]), z.dtype)
        dev_args.append(jax.device_put(cat, sharding))

    out_arrs = sharded(*dev_args)
    jax.block_until_ready(out_arrs)
    times = []
    for _ in range(n_timed):
        t0 = time.perf_counter()
        out_arrs = sharded(*dev_args)
        jax.block_until_ready(out_arrs)
        times.append(time.perf_counter() - t0)

    parts = np.asarray(out_arrs[out_names.index("out_p")]).reshape(NCORES, BC)
    out = (parts.reshape(B) + inputs["b_dec"][0]).astype(np.float32).reshape(B, 1)
    return out, times


def kernel(**inputs):
    out, _ = _run(inputs, T)
    return out
